# revision 18
# baseline (speedup 1.0000x reference)
"""Trainium2 Bass kernel for nn_LoRAExpert (moe_routing).

Per token t (expert e_t from contiguous group_sizes, adapter a_t):

    out[t] = x[t] @ W[e_t] + s_{a_t} * (x[t] @ A[a_t, e_t]) @ B[a_t, e_t]

Strategy (expert-parallel over 8 NeuronCores):
  - Host routes tokens: x is already expert-sorted, so core e gets the
    contiguous slice x[off_e : off_e + gs_e], padded to a common `cap`.
  - LoRA routing trick: with A=8 adapters and rank R=16, the per-expert
    concatenation A_cat = [A[0,e] .. A[7,e]] is [1024, 128]. Compute
    inter_all = x @ A_cat densely for ALL adapters, then multiply by a
    per-token mask M[j, t] = s_{a_t} * (j in adapter-a_t block) and feed
    the masked inter into B_cat = [B[0,e]; ..; B[7,e]] ([128, 1024]).
    This turns the ragged adapter grouping into two dense matmuls and
    one elementwise mask — no on-device sorting or control flow.
  - The B-side matmul accumulates into the same PSUM tile as the base
    matmul, so base + lora is free.
  - All matmul operands are cast to bf16 on the host (fp32 PSUM
    accumulation on the PE); output is written bf16 and widened to fp32
    on the host (absmax rel-err budget is 2e-2; this costs ~3e-3).
  - DMA plan: all inputs go on the Sync HWDGE queue in priority order;
    the output tiles go on the Scalar HWDGE queue so they never block
    late x groups. PSUM->SBUF casts split across Vector (oc0) and
    Scalar (oc1).
  - The base+lora matmuls run as two output-column sweeps: sweep 1
    computes the first 512 output columns of every token tile, sweep 2
    the second 512. Sweep 1 only needs the oc0 half of W (1 MB) at the
    head, at a demand rate (~70 B/ns) far below queue bandwidth, so the
    PE never outruns the input stream; W-oc1 has ~35us to arrive.
  - Phase 1 (x8 @ A8) runs in fp8 DoubleRow (256-row contraction), so
    it depends only on the small fp8 x stream, not the bulk bf16 x.

The kernel is compiled for cap = max(group_sizes) rounded up to 128 and
cached per cap. All 8 cores run one SPMD program; per-core data differs
only through the input maps.
"""

import numpy as np

T, E, IN, OUT, A, R = 16384, 8, 1024, 1024, 8, 16
NCORES = 8
AR = A * R  # 128
KC = IN // 128  # 8 contraction chunks
OC = OUT // 512  # 2 output column chunks

_compiled_cache: dict[int, object] = {}


# ---------------------------------------------------------------------------
# walrus in this container accepts at most 1 sync-wait command per
# instruction; Tile attaches more. Split excess waits onto no-ops.
# ---------------------------------------------------------------------------


def _apply_tile_wait_patch():
    import bass_rust
    import concourse.tile as tile
    from concourse import mybir
    from concourse.vector_clock import ScopedClock

    if getattr(tile.TileContext, "_wait_split_patched", False):
        return

    MAX_WAITS = 1

    def _split_excess_waits(nc):
        for fn in nc.m.functions:
            for blk in fn.blocks:
                insts = blk.instructions  # live list
                i = 0
                while i < len(insts):
                    inst = insts[i]
                    si = inst.sync_info
                    if si is not None and len(si.on_wait) > MAX_WAITS:
                        waits = list(si.on_wait)
                        keep = waits[-MAX_WAITS:]
                        excess = waits[:-MAX_WAITS]
                        inst.sync_info = bass_rust.SyncInfo(
                            on_wait=keep, on_update=list(si.on_update)
                        )
                        pos = i
                        for k in range(0, len(excess), MAX_WAITS):
                            nop = mybir.InstNoOp(
                                name=f"{inst.name}-hoistw{k}",
                                engine=inst.engine,
                                bass_nofuse=True,
                                sync_info=mybir.SyncInfo(
                                    on_wait=excess[k : k + MAX_WAITS], on_update=[]
                                ),
                            )
                            insts.insert(pos, nop)
                            pos += 1
                            i += 1
                    i += 1

    def _split_drain_and_barrier(self, tick_clock, wait_clock):
        nc = self.nc
        drain_inst = nc.sync.drain()
        wait_clock.add_sem_waits(
            drain_inst.ins, ScopedClock({None: tick_clock.global_clock})
        )
        si = drain_inst.ins.sync_info
        if si is not None and len(si.on_wait) > MAX_WAITS:
            waits = list(si.on_wait)
            drain_inst.ins.sync_info = bass_rust.SyncInfo(
                on_wait=waits[:MAX_WAITS], on_update=list(si.on_update)
            )
            for k in range(MAX_WAITS, len(waits), MAX_WAITS):
                extra = nc.sync.drain()
                extra.ins.sync_info = bass_rust.SyncInfo(
                    on_wait=waits[k : k + MAX_WAITS], on_update=[]
                )

        import os as _os

        nc.all_engine_barrier()
        assert self.sems is not None
        popped = nc._tile_sem_poison_stack.pop()
        assert popped is self._sem_poison
        nc.clear_and_free_semaphores(list(self.sems.allocated().values()))
        if _os.environ.get("LORA_LEAN_TAIL", "1") != "1":
            # Second barrier only matters for kernels that continue past
            # the TileContext; ours ends here (sem clears trail on gpsimd).
            nc.all_engine_barrier()

        _split_excess_waits(nc)

    tile.TileContext._drain_and_barrier = _split_drain_and_barrier
    tile.TileContext._wait_split_patched = True


# ---------------------------------------------------------------------------
# Bass program (one SPMD NeuronCore program, parameterized by cap)
# ---------------------------------------------------------------------------


def _build(cap: int):
    import concourse.bass as bass
    import concourse.tile as tile
    from concourse import mybir

    _apply_tile_wait_patch()

    ntt = cap // 128  # token tiles
    ngr = (cap + 511) // 512  # inter groups of up to 512 tokens

    bf16 = mybir.dt.bfloat16
    f32 = mybir.dt.float32

    f8 = mybir.dt.float8e4
    KC2 = KC // 2  # fp8 DoubleRow contracts 256 rows per instruction

    NTB = 512 // 128  # token blocks per group
    TBW = KC * 128  # columns per token block in XT

    nc = bass.Bass()
    # XT[g, p, tb*KC*128 + k*128 + c] = x_e[512g + 128tb + c, 128k + p]
    # (token-block-major so a single token tile's x is one contiguous run)
    XT = nc.dram_tensor("xt", [ngr, 128, NTB * TBW], bf16, kind="ExternalInput")
    # X8[g, p, ((kc*2+i)*512)+c] = fp8(x_e[512g + c, 256kc + 128i + p])
    X8 = nc.dram_tensor("x8", [ngr, 128, KC2 * 2 * 512], f8, kind="ExternalInput")
    # W2[p, oc, k, c] = weight[128k + p, 512oc + c] (oc-major halves)
    W = nc.dram_tensor("w", [128, OC, KC, 512], bf16, kind="ExternalInput")
    A8T = nc.dram_tensor("a8", [128, KC2, 2, AR], f8, kind="ExternalInput")
    BCAT = nc.dram_tensor("bcat", [AR, OUT], bf16, kind="ExternalInput")
    MASKT = nc.dram_tensor("maskt", [AR, cap], bf16, kind="ExternalInput")
    OUTD = nc.dram_tensor("out", [cap, OUT], bf16, kind="ExternalOutput")

    def gslice(g):
        t0 = g * 512
        return t0, min(512, cap - t0)

    with tile.TileContext(nc) as tc:
        with (
            tc.tile_pool(name="big", bufs=1) as big,
            tc.tile_pool(name="outp", bufs=ntt) as outp,
            tc.tile_pool(name="psi", bufs=2, space="PSUM") as psi,
            tc.tile_pool(name="pso", bufs=4, space="PSUM") as pso,
        ):
            # Warm the PE's HAM clock gate during the input-DMA lead-in:
            # ~4us of matmuls on a zeroed tile so real matmuls start at
            # 2.4 GHz instead of 1.2 GHz. Targets a psi bank (all 8 PSUM
            # banks are accounted: 6 pso + 2 psi).
            warm_sb = big.tile([128, 512], bf16)
            nc.vector.memset(warm_sb[:], 0.0)
            wps = psi.tile([128, 512], f32, name="warm", tag="psi")
            for i in range(10):
                nc.tensor.matmul(
                    wps[:], warm_sb[:, 0:128], warm_sb[:],
                    start=(i == 0), stop=(i == 9),
                )
            # All inputs on the Sync HWDGE queue, ordered so each item
            # lands just before the PE needs it in the oc0 sweep; the
            # scalar queue only carries output tiles (emitted later).
            a8_sb = big.tile([128, KC2, 2, AR], f8)
            x8_sb = big.tile([128, ngr, KC2, 2, 512], f8)
            xt_sb = big.tile([128, ngr, NTB, KC, 128], bf16)
            w_sb = big.tile([128, OC, KC, 512], bf16)
            maskt_sb = big.tile([AR, cap], bf16)
            b_sb = big.tile([AR, OUT], bf16)
            nc.sync.dma_start(a8_sb[:], A8T[:])
            nc.sync.dma_start(x8_sb[:, 0, 0:2], X8[0, :, 0 : 2 * 1024])
            nc.sync.dma_start(maskt_sb[:], MASKT[:])
            nc.sync.dma_start(w_sb[:, 0, 0:4, :], W[:, 0, 0:4, :])
            nc.sync.dma_start(xt_sb[:, 0, 0], XT[0, :, 0:TBW])
            nc.sync.dma_start(x8_sb[:, 0, 2:4], X8[0, :, 2 * 1024 : 4 * 1024])
            nc.sync.dma_start(b_sb[:], BCAT[:])
            nc.sync.dma_start(xt_sb[:, 0, 1], XT[0, :, TBW : 2 * TBW])
            nc.sync.dma_start(w_sb[:, 0, 4:8, :], W[:, 0, 4:8, :])
            nc.sync.dma_start(xt_sb[:, 0, 2], XT[0, :, 2 * TBW : 3 * TBW])
            nc.sync.dma_start(xt_sb[:, 0, 3], XT[0, :, 3 * TBW : 4 * TBW])
            if ngr > 1:
                nc.sync.dma_start(x8_sb[:, 1], X8[1])
                nc.sync.dma_start(xt_sb[:, 1, 0:2], XT[1, :, 0 : 2 * TBW])
                nc.sync.dma_start(xt_sb[:, 1, 2:4], XT[1, :, 2 * TBW : 4 * TBW])
            if ngr > 2:
                nc.sync.dma_start(x8_sb[:, 2], X8[2])
                nc.sync.dma_start(xt_sb[:, 2], XT[2])
            nc.sync.dma_start(w_sb[:, 1, 0:4, :], W[:, 1, 0:4, :])
            for g in range(3, ngr):
                nc.sync.dma_start(x8_sb[:, g], X8[g])
                nc.sync.dma_start(xt_sb[:, g], XT[g])
            nc.sync.dma_start(w_sb[:, 1, 4:8, :], W[:, 1, 4:8, :])

            interm_sb = big.tile([AR, cap], bf16)

            def phase1(g):
                # inter_all = (x8 @ A8_cat)^T for group g (fp8 DoubleRow,
                # 256-row contraction per matmul), masked -> interm_sb
                t0, wg = gslice(g)
                ps = psi.tile([128, 512], f32, name=f"psi{g}", tag="psi")
                for kc in range(KC2):
                    nc.tensor.matmul(
                        ps[:, :wg],
                        a8_sb[:, kc],
                        x8_sb[:, g, kc, :, 0:wg],
                        start=(kc == 0),
                        stop=(kc == KC2 - 1),
                        perf_mode=mybir.MatmulPerfMode.DoubleRow,
                    )
                nc.vector.scalar_tensor_tensor(
                    interm_sb[:, t0 : t0 + wg],
                    ps[:, :wg],
                    1.0,
                    maskt_sb[:, t0 : t0 + wg],
                    mybir.AluOpType.mult,
                    mybir.AluOpType.mult,
                )

            o_sbs = {}

            def oc_tile(tt, oc):
                # one token tile's base k-loop for a 512-wide output
                # half, + its lora matmul, cast to SBUF; the out DMA
                # fires once both halves are in the tile's SBUF buffer.
                ts0 = tt * 128
                g, tb = ts0 // 512, (ts0 % 512) // 128
                ps = pso.tile([128, 512], f32, name=f"ps{tt}_{oc}", tag="pso")
                for k in range(KC):
                    nc.tensor.matmul(
                        ps[:],
                        xt_sb[:, g, tb, k, :],
                        w_sb[:, oc, k, :],
                        start=(k == 0),
                        stop=False,
                    )
                nc.tensor.matmul(
                    ps[:],
                    interm_sb[:, ts0 : ts0 + 128],
                    b_sb[:, oc * 512 : oc * 512 + 512],
                    start=False,
                    stop=True,
                )
                if oc == 0:
                    o_sbs[tt] = outp.tile([128, OUT], bf16, name=f"o{tt}", tag="outp")
                    nc.vector.tensor_copy(o_sbs[tt][:, 0:512], ps[:])
                else:
                    nc.scalar.copy(o_sbs[tt][:, 512:1024], ps[:])
                    nc.scalar.dma_start(OUTD[ts0 : ts0 + 128, :], o_sbs[tt][:])

            # Sweep 1: oc0 half of every tile, phase 1 inlined at each
            # group boundary (fp8, so it's paced by the small x8 stream).
            for g in range(ngr):
                phase1(g)
                t0, wg = gslice(g)
                for tt in range(t0 // 128, (t0 + wg) // 128):
                    oc_tile(tt, 0)
            # Sweep 2: oc1 halves; W-oc1 had the whole sweep 1 to land.
            for tt in range(ntt):
                oc_tile(tt, 1)

    return nc


def _get_compiled(cap: int):
    if cap not in _compiled_cache:
        _compiled_cache[cap] = _build(cap)
    return _compiled_cache[cap]


# ---------------------------------------------------------------------------
# Host-side routing + execution
# ---------------------------------------------------------------------------


def _reference_numpy(x, group_sizes, adapter_indices_sorted, weight, lora_A, lora_B, lora_scaling):
    """Fallback replicating the jax reference exactly (only used for
    degenerate group_sizes that do not sum to T)."""
    x = np.asarray(x, np.float32)
    gs = np.asarray(group_sizes, np.int64)
    adapter = np.asarray(adapter_indices_sorted, np.int64)
    out = np.zeros((x.shape[0], weight.shape[2]), np.float32)
    # base: ragged_dot semantics (groups from cumsum, tail rows -> 0)
    offs = np.minimum(np.concatenate([[0], np.cumsum(gs)]), x.shape[0])
    for e in range(E):
        s, t = offs[e], offs[e + 1]
        if t > s:
            out[s:t] = x[s:t] @ weight[e]
    # lora: expert ids via repeat padded with the final value
    rep = np.repeat(np.arange(E), np.maximum(gs, 0))[: x.shape[0]]
    if rep.size == 0:
        rep = np.zeros(x.shape[0], np.int64)
    elif rep.size < x.shape[0]:
        rep = np.concatenate(
            [rep, np.full(x.shape[0] - rep.size, rep[-1], np.int64)]
        )
    for t in range(x.shape[0]):
        e, a = rep[t], adapter[t]
        inter = x[t] @ lora_A[a, e]
        out[t] += lora_scaling[a] * (inter @ lora_B[a, e])
    return out


def kernel(x, group_sizes, adapter_indices_sorted, weight, lora_A, lora_B, lora_scaling):
    import ml_dtypes

    x = np.ascontiguousarray(np.asarray(x, np.float32))
    weight = np.asarray(weight, np.float32)
    lora_A = np.asarray(lora_A, np.float32)
    lora_B = np.asarray(lora_B, np.float32)
    scaling = np.asarray(lora_scaling, np.float32)
    gs = np.asarray(group_sizes).astype(np.int64)
    adapter = np.asarray(adapter_indices_sorted).astype(np.int64)

    if gs.sum() != T or (gs < 0).any():
        return _reference_numpy(
            x, gs, adapter, weight, lora_A, lora_B, scaling
        )

    from concourse.bass_utils import run_bass_kernel_spmd

    bf = ml_dtypes.bfloat16
    f8 = ml_dtypes.float8_e4m3
    cap = int(max(128, -(-int(gs.max()) // 128) * 128))
    nc = _get_compiled(cap)
    KC2 = KC // 2

    offs = np.concatenate([[0], np.cumsum(gs)])
    in_maps = []
    for e in range(NCORES):
        n = int(gs[e])
        s = int(offs[e])
        ngr = (cap + 511) // 512
        xe = np.zeros((ngr * 512, IN), np.float32)
        xe[:n] = x[s : s + n]
        # XT[g, p, tb*KC*128 + k*128 + c] = x_e[512g+128tb+c, 128k+p]
        xt = np.ascontiguousarray(
            xe.reshape(ngr, 4, 128, KC, 128)
            .transpose(0, 4, 1, 3, 2)
            .reshape(ngr, 128, 4 * KC * 128)
            .astype(bf)
        )
        # X8[g, p, (kc*2+i)*512+c] = fp8(x_e[512g+c, 256kc+128i+p])
        x8 = np.ascontiguousarray(
            xe.reshape(ngr, 512, KC2, 2, 128)
            .transpose(0, 4, 2, 3, 1)
            .reshape(ngr, 128, KC2 * 2 * 512)
            .astype(f8)
        )
        # W2[p, oc, k, c] = weight[e][128k+p, 512oc+c]
        w = np.ascontiguousarray(
            weight[e].reshape(KC, 128, OC, 512).transpose(1, 2, 0, 3).astype(bf)
        )
        # A_cat[:, a*R+r] = lora_A[a, e, :, r]; A8[p, kc, i, j] =
        # fp8(A_cat[256kc+128i+p, j])
        acat_full = lora_A[:, e].transpose(1, 0, 2).reshape(IN, AR)
        a8 = np.ascontiguousarray(
            acat_full.reshape(KC2, 2, 128, AR).transpose(2, 0, 1, 3).astype(f8)
        )
        bcat = np.ascontiguousarray(lora_B[:, e].reshape(AR, OUT).astype(bf))
        ae = adapter[s : s + n]
        m = np.zeros((A, cap), np.float32)
        m[ae, np.arange(n)] = scaling[ae]
        maskt = np.ascontiguousarray(np.repeat(m, R, axis=0).astype(bf))
        in_maps.append(
            {"xt": xt, "x8": x8, "w": w, "a8": a8, "bcat": bcat, "maskt": maskt}
        )

    res = run_bass_kernel_spmd(nc, in_maps, list(range(NCORES)))

    out = np.empty((T, OUT), np.float32)
    for e in range(NCORES):
        n = int(gs[e])
        if n:
            out[int(offs[e]) : int(offs[e]) + n] = (
                res.results[e]["out"][:n].astype(np.float32)
            )
    return out


# revision 21
# speedup vs baseline: 1.1273x; 1.1273x over previous
"""Trainium2 Bass kernel for nn_LoRAExpert (moe_routing).

Per token t (expert e_t from contiguous group_sizes, adapter a_t):

    out[t] = x[t] @ W[e_t] + s_{a_t} * (x[t] @ A[a_t, e_t]) @ B[a_t, e_t]

Strategy (expert-parallel over 8 NeuronCores):
  - Host routes tokens: x is already expert-sorted, so core e gets the
    contiguous slice x[off_e : off_e + gs_e], padded to a common `cap`.
  - LoRA routing trick: with A=8 adapters and rank R=16, the per-expert
    concatenation A_cat = [A[0,e] .. A[7,e]] is [1024, 128]. Compute
    inter_all = x @ A_cat densely for ALL adapters, then multiply by a
    per-token mask M[j, t] = s_{a_t} * (j in adapter-a_t block) and feed
    the masked inter into B_cat = [B[0,e]; ..; B[7,e]] ([128, 1024]).
    This turns the ragged adapter grouping into two dense matmuls and
    one elementwise mask — no on-device sorting or control flow.
  - The B-side matmul accumulates into the same PSUM tile as the base
    matmul, so base + lora is free.
  - All matmul operands are cast to bf16 on the host (fp32 PSUM
    accumulation on the PE); output is written bf16 and widened to fp32
    on the host (absmax rel-err budget is 2e-2; this costs ~3e-3).
  - DMA plan: all inputs go on the Sync HWDGE queue in priority order;
    the output tiles go on the Scalar HWDGE queue so they never block
    late x groups. PSUM->SBUF casts split across Vector (oc0) and
    Scalar (oc1).
  - The base+lora matmuls run as two output-column sweeps: sweep 1
    computes the first 512 output columns of every token tile, sweep 2
    the second 512. Sweep 1 only needs the oc0 half of W (1 MB) at the
    head, at a demand rate (~70 B/ns) far below queue bandwidth, so the
    PE never outruns the input stream; W-oc1 has ~35us to arrive.
  - Phase 1 (x8 @ A8) runs in fp8 DoubleRow (256-row contraction), so
    it depends only on the small fp8 x stream, not the bulk bf16 x.

The kernel is compiled for cap = max(group_sizes) rounded up to 128 and
cached per cap. All 8 cores run one SPMD program; per-core data differs
only through the input maps.
"""

import numpy as np

T, E, IN, OUT, A, R = 16384, 8, 1024, 1024, 8, 16
NCORES = 8
AR = A * R  # 128
KC = IN // 128  # 8 contraction chunks
OC = OUT // 512  # 2 output column chunks

_compiled_cache: dict[int, object] = {}


# ---------------------------------------------------------------------------
# walrus in this container accepts at most 1 sync-wait command per
# instruction; Tile attaches more. Split excess waits onto no-ops.
# ---------------------------------------------------------------------------


def _apply_tile_wait_patch():
    import bass_rust
    import concourse.tile as tile
    from concourse import mybir
    from concourse.vector_clock import ScopedClock

    if getattr(tile.TileContext, "_wait_split_patched", False):
        return

    MAX_WAITS = 1

    def _split_excess_waits(nc):
        for fn in nc.m.functions:
            for blk in fn.blocks:
                insts = blk.instructions  # live list
                i = 0
                while i < len(insts):
                    inst = insts[i]
                    si = inst.sync_info
                    if si is not None and len(si.on_wait) > MAX_WAITS:
                        waits = list(si.on_wait)
                        keep = waits[-MAX_WAITS:]
                        excess = waits[:-MAX_WAITS]
                        inst.sync_info = bass_rust.SyncInfo(
                            on_wait=keep, on_update=list(si.on_update)
                        )
                        pos = i
                        for k in range(0, len(excess), MAX_WAITS):
                            nop = mybir.InstNoOp(
                                name=f"{inst.name}-hoistw{k}",
                                engine=inst.engine,
                                bass_nofuse=True,
                                sync_info=mybir.SyncInfo(
                                    on_wait=excess[k : k + MAX_WAITS], on_update=[]
                                ),
                            )
                            insts.insert(pos, nop)
                            pos += 1
                            i += 1
                    i += 1

    def _split_drain_and_barrier(self, tick_clock, wait_clock):
        nc = self.nc
        drain_inst = nc.sync.drain()
        wait_clock.add_sem_waits(
            drain_inst.ins, ScopedClock({None: tick_clock.global_clock})
        )
        si = drain_inst.ins.sync_info
        if si is not None and len(si.on_wait) > MAX_WAITS:
            waits = list(si.on_wait)
            drain_inst.ins.sync_info = bass_rust.SyncInfo(
                on_wait=waits[:MAX_WAITS], on_update=list(si.on_update)
            )
            for k in range(MAX_WAITS, len(waits), MAX_WAITS):
                extra = nc.sync.drain()
                extra.ins.sync_info = bass_rust.SyncInfo(
                    on_wait=waits[k : k + MAX_WAITS], on_update=[]
                )

        import os as _os

        nc.all_engine_barrier()
        assert self.sems is not None
        popped = nc._tile_sem_poison_stack.pop()
        assert popped is self._sem_poison
        nc.clear_and_free_semaphores(list(self.sems.allocated().values()))
        if _os.environ.get("LORA_LEAN_TAIL", "1") != "1":
            # Second barrier only matters for kernels that continue past
            # the TileContext; ours ends here (sem clears trail on gpsimd).
            nc.all_engine_barrier()

        _split_excess_waits(nc)

    tile.TileContext._drain_and_barrier = _split_drain_and_barrier
    tile.TileContext._wait_split_patched = True


# ---------------------------------------------------------------------------
# Bass program (one SPMD NeuronCore program, parameterized by cap)
# ---------------------------------------------------------------------------


def _build(cap: int):
    import concourse.bass as bass
    import concourse.tile as tile
    from concourse import mybir

    _apply_tile_wait_patch()

    ntt = cap // 128  # token tiles
    ngr = (cap + 511) // 512  # inter groups of up to 512 tokens

    bf16 = mybir.dt.bfloat16
    f32 = mybir.dt.float32

    f8 = mybir.dt.float8e4
    KC2 = KC // 2  # fp8 DoubleRow contracts 256 rows per instruction

    NTB = 512 // 128  # token blocks per group
    TBW = KC * 128  # columns per token block in XT

    nc = bass.Bass()
    # XT[g, p, tb*KC*128 + k*128 + c] = x_e[512g + 128tb + c, 128k + p]
    # (token-block-major so a single token tile's x is one contiguous run)
    XT = nc.dram_tensor("xt", [ngr, 128, NTB * TBW], bf16, kind="ExternalInput")
    # X8[g, p, ((kc*2+i)*512)+c] = fp8(x_e[512g + c, 256kc + 128i + p])
    X8 = nc.dram_tensor("x8", [ngr, 128, KC2 * 2 * 512], f8, kind="ExternalInput")
    # W2[p, oc, k, c] = weight[128k + p, 512oc + c] (oc-major halves)
    W = nc.dram_tensor("w", [128, OC, KC, 512], bf16, kind="ExternalInput")
    A8T = nc.dram_tensor("a8", [128, KC2, 2, AR], f8, kind="ExternalInput")
    BCAT = nc.dram_tensor("bcat", [AR, OUT], bf16, kind="ExternalInput")
    MASKT = nc.dram_tensor("maskt", [AR, cap], bf16, kind="ExternalInput")
    OUTD = nc.dram_tensor("out", [cap, OUT], bf16, kind="ExternalOutput")

    def gslice(g):
        t0 = g * 512
        return t0, min(512, cap - t0)

    with tile.TileContext(nc) as tc:
        with (
            tc.tile_pool(name="big", bufs=1) as big,
            tc.tile_pool(name="outp", bufs=4) as outp,
            tc.tile_pool(name="psi", bufs=2, space="PSUM") as psi,
            tc.tile_pool(name="pso", bufs=4, space="PSUM") as pso,
        ):
            # Warm the PE's HAM clock gate during the input-DMA lead-in:
            # ~4us of matmuls on a zeroed tile so real matmuls start at
            # 2.4 GHz instead of 1.2 GHz. Targets a psi bank (all 8 PSUM
            # banks are accounted: 6 pso + 2 psi).
            warm_sb = big.tile([128, 512], bf16)
            nc.vector.memset(warm_sb[:], 0.0)
            wps = psi.tile([128, 512], f32, name="warm", tag="psi")
            for i in range(10):
                nc.tensor.matmul(
                    wps[:], warm_sb[:, 0:128], warm_sb[:],
                    start=(i == 0), stop=(i == 9),
                )
            # All inputs on the Sync HWDGE queue, ordered so each item
            # lands just before the PE needs it in the oc0 sweep; the
            # scalar queue only carries output tiles (emitted later).
            a8_sb = big.tile([128, KC2, 2, AR], f8)
            x8_sb = big.tile([128, ngr, KC2, 2, 512], f8)
            xt_sb = big.tile([128, ngr, NTB, KC, 128], bf16)
            w_sb = big.tile([128, OC, KC, 512], bf16)
            maskt_sb = big.tile([AR, cap], bf16)
            b_sb = big.tile([AR, OUT], bf16)
            nc.sync.dma_start(a8_sb[:], A8T[:])
            nc.sync.dma_start(x8_sb[:, 0, 0:2], X8[0, :, 0 : 2 * 1024])
            nc.sync.dma_start(x8_sb[:, 0, 2:4], X8[0, :, 2 * 1024 : 4 * 1024])
            nc.sync.dma_start(w_sb[:, 0, 0:2, :], W[:, 0, 0:2, :])
            nc.sync.dma_start(xt_sb[:, 0, 0], XT[0, :, 0:TBW])
            nc.sync.dma_start(xt_sb[:, 0, 1], XT[0, :, TBW : 2 * TBW])
            nc.sync.dma_start(w_sb[:, 0, 2:4, :], W[:, 0, 2:4, :])
            nc.sync.dma_start(w_sb[:, 0, 4:6, :], W[:, 0, 4:6, :])
            nc.sync.dma_start(w_sb[:, 0, 6:8, :], W[:, 0, 6:8, :])
            nc.sync.dma_start(maskt_sb[:], MASKT[:])
            nc.sync.dma_start(b_sb[:], BCAT[:])
            nc.sync.dma_start(xt_sb[:, 0, 2], XT[0, :, 2 * TBW : 3 * TBW])
            nc.sync.dma_start(xt_sb[:, 0, 3], XT[0, :, 3 * TBW : 4 * TBW])
            if ngr > 1:
                nc.sync.dma_start(xt_sb[:, 1, 0:2], XT[1, :, 0 : 2 * TBW])
                nc.sync.dma_start(x8_sb[:, 1], X8[1])
                nc.sync.dma_start(xt_sb[:, 1, 2:4], XT[1, :, 2 * TBW : 4 * TBW])
            if ngr > 2:
                nc.sync.dma_start(x8_sb[:, 2], X8[2])
                nc.sync.dma_start(xt_sb[:, 2], XT[2])
            nc.sync.dma_start(w_sb[:, 1, 0:4, :], W[:, 1, 0:4, :])
            for g in range(3, ngr):
                nc.sync.dma_start(x8_sb[:, g], X8[g])
                nc.sync.dma_start(xt_sb[:, g], XT[g])
            nc.sync.dma_start(w_sb[:, 1, 4:8, :], W[:, 1, 4:8, :])

            interm_sb = big.tile([AR, cap], bf16)

            def phase1(g):
                # inter_all = (x8 @ A8_cat)^T for group g (fp8 DoubleRow,
                # 256-row contraction per matmul), masked -> interm_sb
                t0, wg = gslice(g)
                ps = psi.tile([128, 512], f32, name=f"psi{g}", tag="psi")
                for kc in range(KC2):
                    nc.tensor.matmul(
                        ps[:, :wg],
                        a8_sb[:, kc],
                        x8_sb[:, g, kc, :, 0:wg],
                        start=(kc == 0),
                        stop=(kc == KC2 - 1),
                        perf_mode=mybir.MatmulPerfMode.DoubleRow,
                    )
                nc.vector.scalar_tensor_tensor(
                    interm_sb[:, t0 : t0 + wg],
                    ps[:, :wg],
                    1.0,
                    maskt_sb[:, t0 : t0 + wg],
                    mybir.AluOpType.mult,
                    mybir.AluOpType.mult,
                )

            def base_pair(tts, oc):
                # base k-loops for up to two token tiles, interleaved so
                # consecutive matmuls hit different PSUM banks (same-bank
                # back-to-back accumulation costs ~24ns/matmul).
                pss = [
                    pso.tile([128, 512], f32, name=f"ps{tt}_{oc}", tag="pso")
                    for tt in tts
                ]
                for k in range(KC):
                    for ps, tt in zip(pss, tts):
                        ts0 = tt * 128
                        g, tb = ts0 // 512, (ts0 % 512) // 128
                        nc.tensor.matmul(
                            ps[:],
                            xt_sb[:, g, tb, k, :],
                            w_sb[:, oc, k, :],
                            start=(k == 0),
                            stop=False,
                        )
                return pss

            def lora_out_pair(tts, pss, oc):
                # lora matmuls for the pair, then cast + per-half DMA out.
                for ps, tt in zip(pss, tts):
                    ts0 = tt * 128
                    nc.tensor.matmul(
                        ps[:],
                        interm_sb[:, ts0 : ts0 + 128],
                        b_sb[:, oc * 512 : oc * 512 + 512],
                        start=False,
                        stop=True,
                    )
                for ps, tt in zip(pss, tts):
                    ts0 = tt * 128
                    o_sb = outp.tile([128, 512], bf16, name=f"o{tt}_{oc}", tag="outp")
                    if oc == 0:
                        nc.vector.tensor_copy(o_sb[:], ps[:])
                    else:
                        nc.scalar.copy(o_sb[:], ps[:])
                    nc.scalar.dma_start(
                        OUTD[ts0 : ts0 + 128, oc * 512 : oc * 512 + 512], o_sb[:]
                    )

            def sweep(oc):
                # software-pipelined: pair i's lora/copy runs after pair
                # i+1's base k-loop, giving mask/B/STT extra slack.
                pending = None
                for g in range(ngr) if oc == 0 else [None]:
                    if oc == 0:
                        phase1(g)
                        t0, wg = gslice(g)
                        tts = list(range(t0 // 128, (t0 + wg) // 128))
                    else:
                        tts = list(range(ntt))
                    for i in range(0, len(tts), 2):
                        pair = tts[i : i + 2]
                        pss = base_pair(pair, oc)
                        if pending is not None:
                            lora_out_pair(*pending, oc)
                        pending = (pair, pss)
                if pending is not None:
                    lora_out_pair(*pending, oc)

            # Sweep 1: oc0 half of every tile (needs only W-oc0 early);
            # Sweep 2: oc1 halves — W-oc1 had all of sweep 1 to land.
            sweep(0)
            sweep(1)

    return nc


def _get_compiled(cap: int):
    if cap not in _compiled_cache:
        _compiled_cache[cap] = _build(cap)
    return _compiled_cache[cap]


# ---------------------------------------------------------------------------
# Host-side routing + execution
# ---------------------------------------------------------------------------


def _reference_numpy(x, group_sizes, adapter_indices_sorted, weight, lora_A, lora_B, lora_scaling):
    """Fallback replicating the jax reference exactly (only used for
    degenerate group_sizes that do not sum to T)."""
    x = np.asarray(x, np.float32)
    gs = np.asarray(group_sizes, np.int64)
    adapter = np.asarray(adapter_indices_sorted, np.int64)
    out = np.zeros((x.shape[0], weight.shape[2]), np.float32)
    # base: ragged_dot semantics (groups from cumsum, tail rows -> 0)
    offs = np.minimum(np.concatenate([[0], np.cumsum(gs)]), x.shape[0])
    for e in range(E):
        s, t = offs[e], offs[e + 1]
        if t > s:
            out[s:t] = x[s:t] @ weight[e]
    # lora: expert ids via repeat padded with the final value
    rep = np.repeat(np.arange(E), np.maximum(gs, 0))[: x.shape[0]]
    if rep.size == 0:
        rep = np.zeros(x.shape[0], np.int64)
    elif rep.size < x.shape[0]:
        rep = np.concatenate(
            [rep, np.full(x.shape[0] - rep.size, rep[-1], np.int64)]
        )
    for t in range(x.shape[0]):
        e, a = rep[t], adapter[t]
        inter = x[t] @ lora_A[a, e]
        out[t] += lora_scaling[a] * (inter @ lora_B[a, e])
    return out


def kernel(x, group_sizes, adapter_indices_sorted, weight, lora_A, lora_B, lora_scaling):
    import ml_dtypes

    x = np.ascontiguousarray(np.asarray(x, np.float32))
    weight = np.asarray(weight, np.float32)
    lora_A = np.asarray(lora_A, np.float32)
    lora_B = np.asarray(lora_B, np.float32)
    scaling = np.asarray(lora_scaling, np.float32)
    gs = np.asarray(group_sizes).astype(np.int64)
    adapter = np.asarray(adapter_indices_sorted).astype(np.int64)

    if gs.sum() != T or (gs < 0).any():
        return _reference_numpy(
            x, gs, adapter, weight, lora_A, lora_B, scaling
        )

    from concourse.bass_utils import run_bass_kernel_spmd

    bf = ml_dtypes.bfloat16
    f8 = ml_dtypes.float8_e4m3
    cap = int(max(128, -(-int(gs.max()) // 128) * 128))
    nc = _get_compiled(cap)
    KC2 = KC // 2

    offs = np.concatenate([[0], np.cumsum(gs)])
    in_maps = []
    for e in range(NCORES):
        n = int(gs[e])
        s = int(offs[e])
        ngr = (cap + 511) // 512
        xe = np.zeros((ngr * 512, IN), np.float32)
        xe[:n] = x[s : s + n]
        # XT[g, p, tb*KC*128 + k*128 + c] = x_e[512g+128tb+c, 128k+p]
        xt = np.ascontiguousarray(
            xe.reshape(ngr, 4, 128, KC, 128)
            .transpose(0, 4, 1, 3, 2)
            .reshape(ngr, 128, 4 * KC * 128)
            .astype(bf)
        )
        # X8[g, p, (kc*2+i)*512+c] = fp8(x_e[512g+c, 256kc+128i+p])
        x8 = np.ascontiguousarray(
            xe.reshape(ngr, 512, KC2, 2, 128)
            .transpose(0, 4, 2, 3, 1)
            .reshape(ngr, 128, KC2 * 2 * 512)
            .astype(f8)
        )
        # W2[p, oc, k, c] = weight[e][128k+p, 512oc+c]
        w = np.ascontiguousarray(
            weight[e].reshape(KC, 128, OC, 512).transpose(1, 2, 0, 3).astype(bf)
        )
        # A_cat[:, a*R+r] = lora_A[a, e, :, r]; A8[p, kc, i, j] =
        # fp8(A_cat[256kc+128i+p, j])
        acat_full = lora_A[:, e].transpose(1, 0, 2).reshape(IN, AR)
        a8 = np.ascontiguousarray(
            acat_full.reshape(KC2, 2, 128, AR).transpose(2, 0, 1, 3).astype(f8)
        )
        bcat = np.ascontiguousarray(lora_B[:, e].reshape(AR, OUT).astype(bf))
        ae = adapter[s : s + n]
        m = np.zeros((A, cap), np.float32)
        m[ae, np.arange(n)] = scaling[ae]
        maskt = np.ascontiguousarray(np.repeat(m, R, axis=0).astype(bf))
        in_maps.append(
            {"xt": xt, "x8": x8, "w": w, "a8": a8, "bcat": bcat, "maskt": maskt}
        )

    res = run_bass_kernel_spmd(nc, in_maps, list(range(NCORES)))

    out = np.empty((T, OUT), np.float32)
    for e in range(NCORES):
        n = int(gs[e])
        if n:
            out[int(offs[e]) : int(offs[e]) + n] = (
                res.results[e]["out"][:n].astype(np.float32)
            )
    return out


# revision 24
# speedup vs baseline: 1.1462x; 1.0168x over previous
"""Trainium2 Bass kernel for nn_LoRAExpert (moe_routing).

Per token t (expert e_t from contiguous group_sizes, adapter a_t):

    out[t] = x[t] @ W[e_t] + s_{a_t} * (x[t] @ A[a_t, e_t]) @ B[a_t, e_t]

Strategy (expert-parallel over 8 NeuronCores):
  - Host routes tokens: x is already expert-sorted, so core e gets the
    contiguous slice x[off_e : off_e + gs_e], padded to a common `cap`.
  - LoRA routing trick: with A=8 adapters and rank R=16, the per-expert
    concatenation A_cat = [A[0,e] .. A[7,e]] is [1024, 128]. Compute
    inter_all = x @ A_cat densely for ALL adapters, then multiply by a
    per-token mask M[j, t] = s_{a_t} * (j in adapter-a_t block) and feed
    the masked inter into B_cat = [B[0,e]; ..; B[7,e]] ([128, 1024]).
    This turns the ragged adapter grouping into two dense matmuls and
    one elementwise mask — no on-device sorting or control flow.
  - The B-side matmul accumulates into the same PSUM tile as the base
    matmul, so base + lora is free.
  - All matmul operands are cast to bf16 on the host (fp32 PSUM
    accumulation on the PE); output is written bf16 and widened to fp32
    on the host (absmax rel-err budget is 2e-2; this costs ~3e-3).
  - DMA plan: all inputs go on the Sync HWDGE queue in priority order;
    the output tiles go on the Scalar HWDGE queue so they never block
    late x groups. PSUM->SBUF casts split across Vector (oc0) and
    Scalar (oc1).
  - The base+lora matmuls run as two output-column sweeps: sweep 1
    computes the first 512 output columns of every token tile, sweep 2
    the second 512. Sweep 1 only needs the oc0 half of W (1 MB) at the
    head, at a demand rate (~70 B/ns) far below queue bandwidth, so the
    PE never outruns the input stream; W-oc1 has ~35us to arrive.
  - Phase 1 (x8 @ A8) runs in fp8 DoubleRow (256-row contraction), so
    it depends only on the small fp8 x stream, not the bulk bf16 x.

The kernel is compiled for cap = max(group_sizes) rounded up to 128 and
cached per cap. All 8 cores run one SPMD program; per-core data differs
only through the input maps.
"""

import numpy as np

T, E, IN, OUT, A, R = 16384, 8, 1024, 1024, 8, 16
NCORES = 8
AR = A * R  # 128
KC = IN // 128  # 8 contraction chunks
OC = OUT // 512  # 2 output column chunks

_compiled_cache: dict[int, object] = {}


# ---------------------------------------------------------------------------
# walrus in this container accepts at most 1 sync-wait command per
# instruction; Tile attaches more. Split excess waits onto no-ops.
# ---------------------------------------------------------------------------


def _apply_tile_wait_patch():
    import bass_rust
    import concourse.tile as tile
    from concourse import mybir
    from concourse.vector_clock import ScopedClock

    if getattr(tile.TileContext, "_wait_split_patched", False):
        return

    MAX_WAITS = 1

    def _split_excess_waits(nc):
        for fn in nc.m.functions:
            for blk in fn.blocks:
                insts = blk.instructions  # live list
                i = 0
                while i < len(insts):
                    inst = insts[i]
                    si = inst.sync_info
                    if si is not None and len(si.on_wait) > MAX_WAITS:
                        waits = list(si.on_wait)
                        keep = waits[-MAX_WAITS:]
                        excess = waits[:-MAX_WAITS]
                        inst.sync_info = bass_rust.SyncInfo(
                            on_wait=keep, on_update=list(si.on_update)
                        )
                        pos = i
                        for k in range(0, len(excess), MAX_WAITS):
                            nop = mybir.InstNoOp(
                                name=f"{inst.name}-hoistw{k}",
                                engine=inst.engine,
                                bass_nofuse=True,
                                sync_info=mybir.SyncInfo(
                                    on_wait=excess[k : k + MAX_WAITS], on_update=[]
                                ),
                            )
                            insts.insert(pos, nop)
                            pos += 1
                            i += 1
                    i += 1

    def _split_drain_and_barrier(self, tick_clock, wait_clock):
        nc = self.nc
        drain_inst = nc.sync.drain()
        wait_clock.add_sem_waits(
            drain_inst.ins, ScopedClock({None: tick_clock.global_clock})
        )
        si = drain_inst.ins.sync_info
        if si is not None and len(si.on_wait) > MAX_WAITS:
            waits = list(si.on_wait)
            drain_inst.ins.sync_info = bass_rust.SyncInfo(
                on_wait=waits[:MAX_WAITS], on_update=list(si.on_update)
            )
            for k in range(MAX_WAITS, len(waits), MAX_WAITS):
                extra = nc.sync.drain()
                extra.ins.sync_info = bass_rust.SyncInfo(
                    on_wait=waits[k : k + MAX_WAITS], on_update=[]
                )

        import os as _os

        nc.all_engine_barrier()
        assert self.sems is not None
        popped = nc._tile_sem_poison_stack.pop()
        assert popped is self._sem_poison
        nc.clear_and_free_semaphores(list(self.sems.allocated().values()))
        if _os.environ.get("LORA_LEAN_TAIL", "1") != "1":
            # Second barrier only matters for kernels that continue past
            # the TileContext; ours ends here (sem clears trail on gpsimd).
            nc.all_engine_barrier()

        _split_excess_waits(nc)

    tile.TileContext._drain_and_barrier = _split_drain_and_barrier
    tile.TileContext._wait_split_patched = True


# ---------------------------------------------------------------------------
# Bass program (one SPMD NeuronCore program, parameterized by cap)
# ---------------------------------------------------------------------------


def _build(cap: int):
    import concourse.bass as bass
    import concourse.tile as tile
    from concourse import mybir

    _apply_tile_wait_patch()

    ntt = cap // 128  # token tiles
    ngr = (cap + 511) // 512  # inter groups of up to 512 tokens

    bf16 = mybir.dt.bfloat16
    f32 = mybir.dt.float32

    f8 = mybir.dt.float8e4
    KC2 = KC // 2  # fp8 DoubleRow contracts 256 rows per instruction

    NTB = 512 // 128  # token blocks per group
    TBW = KC * 128  # columns per token block in XT

    nc = bass.Bass()
    # XT[g, p, tb*KC*128 + k*128 + c] = x_e[512g + 128tb + c, 128k + p]
    # (token-block-major so a single token tile's x is one contiguous run)
    XT = nc.dram_tensor("xt", [ngr, 128, NTB * TBW], bf16, kind="ExternalInput")
    # X8[g, p, ((kc*2+i)*512)+c] = fp8(x_e[512g + c, 256kc + 128i + p])
    X8 = nc.dram_tensor("x8", [ngr, 128, KC2 * 2 * 512], f8, kind="ExternalInput")
    # W2[p, oc, k, c] = weight[128k + p, 512oc + c] (oc-major halves)
    W = nc.dram_tensor("w", [128, OC, KC, 512], bf16, kind="ExternalInput")
    A8T = nc.dram_tensor("a8", [128, KC2, 2, AR], f8, kind="ExternalInput")
    BCAT = nc.dram_tensor("bcat", [AR, OUT], bf16, kind="ExternalInput")
    MASKT = nc.dram_tensor("maskt", [AR, cap], bf16, kind="ExternalInput")
    OUTD = nc.dram_tensor("out", [cap, OUT], bf16, kind="ExternalOutput")

    def gslice(g):
        t0 = g * 512
        return t0, min(512, cap - t0)

    with tile.TileContext(nc) as tc:
        with (
            tc.tile_pool(name="big", bufs=1) as big,
            tc.tile_pool(name="outp", bufs=4) as outp,
            tc.tile_pool(name="psi", bufs=2, space="PSUM") as psi,
            tc.tile_pool(name="pso", bufs=4, space="PSUM") as pso,
        ):
            # Warm the PE's HAM clock gate during the input-DMA lead-in:
            # ~4us of matmuls on a zeroed tile so real matmuls start at
            # 2.4 GHz instead of 1.2 GHz. Targets a psi bank (all 8 PSUM
            # banks are accounted: 6 pso + 2 psi).
            warm_sb = big.tile([128, 512], bf16)
            nc.vector.memset(warm_sb[:], 0.0)
            wps = psi.tile([128, 512], f32, name="warm", tag="psi")
            NWARM = 16  # sized to span the head input-DMA window so the
            for i in range(NWARM):  # clock is fully ramped at first real work
                nc.tensor.matmul(
                    wps[:], warm_sb[:, 0:128], warm_sb[:],
                    start=(i == 0), stop=(i == NWARM - 1),
                )
            # All inputs on the Sync HWDGE queue, ordered so each item
            # lands just before the PE needs it in the oc0 sweep; the
            # scalar queue only carries output tiles (emitted later).
            a8_sb = big.tile([128, KC2, 2, AR], f8)
            x8_sb = big.tile([128, ngr, KC2, 2, 512], f8)
            xt_sb = big.tile([128, ngr, NTB, KC, 128], bf16)
            w_sb = big.tile([128, OC, KC, 512], bf16)
            maskt_sb = big.tile([AR, cap], bf16)
            b_sb = big.tile([AR, OUT], bf16)
            nc.sync.dma_start(a8_sb[:], A8T[:])
            nc.sync.dma_start(x8_sb[:, 0, 0:2], X8[0, :, 0 : 2 * 1024])
            nc.sync.dma_start(x8_sb[:, 0, 2:4], X8[0, :, 2 * 1024 : 4 * 1024])
            nc.sync.dma_start(w_sb[:, 0, 0:2, :], W[:, 0, 0:2, :])
            nc.sync.dma_start(xt_sb[:, 0, 0], XT[0, :, 0:TBW])
            nc.sync.dma_start(xt_sb[:, 0, 1], XT[0, :, TBW : 2 * TBW])
            nc.sync.dma_start(w_sb[:, 0, 2:4, :], W[:, 0, 2:4, :])
            nc.sync.dma_start(w_sb[:, 0, 4:6, :], W[:, 0, 4:6, :])
            nc.sync.dma_start(w_sb[:, 0, 6:8, :], W[:, 0, 6:8, :])
            nc.sync.dma_start(maskt_sb[:], MASKT[:])
            nc.sync.dma_start(b_sb[:], BCAT[:])
            nc.sync.dma_start(xt_sb[:, 0, 2], XT[0, :, 2 * TBW : 3 * TBW])
            nc.sync.dma_start(xt_sb[:, 0, 3], XT[0, :, 3 * TBW : 4 * TBW])
            if ngr > 1:
                nc.sync.dma_start(xt_sb[:, 1, 0:2], XT[1, :, 0 : 2 * TBW])
                nc.sync.dma_start(x8_sb[:, 1], X8[1])
                nc.sync.dma_start(xt_sb[:, 1, 2:4], XT[1, :, 2 * TBW : 4 * TBW])
            if ngr > 2:
                nc.sync.dma_start(x8_sb[:, 2], X8[2])
                nc.sync.dma_start(xt_sb[:, 2], XT[2])
            # W-oc1 pieces interleaved mid-queue: needed from sweep 2
            # (~2/3 into the run) but must not sit behind all x groups.
            nc.sync.dma_start(w_sb[:, 1, 0:2, :], W[:, 1, 0:2, :])
            if ngr > 3:
                nc.sync.dma_start(x8_sb[:, 3], X8[3])
                nc.sync.dma_start(xt_sb[:, 3], XT[3])
            nc.sync.dma_start(w_sb[:, 1, 2:4, :], W[:, 1, 2:4, :])
            nc.sync.dma_start(w_sb[:, 1, 4:6, :], W[:, 1, 4:6, :])
            for g in range(4, ngr):
                nc.sync.dma_start(x8_sb[:, g], X8[g])
                nc.sync.dma_start(xt_sb[:, g], XT[g])
            nc.sync.dma_start(w_sb[:, 1, 6:8, :], W[:, 1, 6:8, :])

            interm_sb = big.tile([AR, cap], bf16)

            def phase1(g):
                # inter_all = (x8 @ A8_cat)^T for group g (fp8 DoubleRow,
                # 256-row contraction per matmul), masked -> interm_sb
                t0, wg = gslice(g)
                ps = psi.tile([128, 512], f32, name=f"psi{g}", tag="psi")
                for kc in range(KC2):
                    nc.tensor.matmul(
                        ps[:, :wg],
                        a8_sb[:, kc],
                        x8_sb[:, g, kc, :, 0:wg],
                        start=(kc == 0),
                        stop=(kc == KC2 - 1),
                        perf_mode=mybir.MatmulPerfMode.DoubleRow,
                    )
                nc.vector.scalar_tensor_tensor(
                    interm_sb[:, t0 : t0 + wg],
                    ps[:, :wg],
                    1.0,
                    maskt_sb[:, t0 : t0 + wg],
                    mybir.AluOpType.mult,
                    mybir.AluOpType.mult,
                )

            def base_pair(tts, oc):
                # base k-loops for up to two token tiles, interleaved so
                # consecutive matmuls hit different PSUM banks (same-bank
                # back-to-back accumulation costs ~24ns/matmul).
                pss = [
                    pso.tile([128, 512], f32, name=f"ps{tt}_{oc}", tag="pso")
                    for tt in tts
                ]
                for k in range(KC):
                    for ps, tt in zip(pss, tts):
                        ts0 = tt * 128
                        g, tb = ts0 // 512, (ts0 % 512) // 128
                        nc.tensor.matmul(
                            ps[:],
                            xt_sb[:, g, tb, k, :],
                            w_sb[:, oc, k, :],
                            start=(k == 0),
                            stop=False,
                        )
                return pss

            def lora_out_pair(tts, pss, oc):
                # lora matmuls for the pair, then cast + per-half DMA out.
                for ps, tt in zip(pss, tts):
                    ts0 = tt * 128
                    nc.tensor.matmul(
                        ps[:],
                        interm_sb[:, ts0 : ts0 + 128],
                        b_sb[:, oc * 512 : oc * 512 + 512],
                        start=False,
                        stop=True,
                    )
                for ps, tt in zip(pss, tts):
                    ts0 = tt * 128
                    o_sb = outp.tile([128, 512], bf16, name=f"o{tt}_{oc}", tag="outp")
                    # alternate cast engines by tile parity so a pair's
                    # two casts run concurrently (vector + scalar)
                    if tt % 2 == 0:
                        nc.vector.tensor_copy(o_sb[:], ps[:])
                    else:
                        nc.scalar.copy(o_sb[:], ps[:])
                    nc.scalar.dma_start(
                        OUTD[ts0 : ts0 + 128, oc * 512 : oc * 512 + 512], o_sb[:]
                    )

            def sweep(oc):
                # software-pipelined: pair i's lora/copy runs after pair
                # i+1's base k-loop, giving mask/B/STT extra slack.
                pending = None
                for g in range(ngr) if oc == 0 else [None]:
                    if oc == 0:
                        phase1(g)
                        t0, wg = gslice(g)
                        tts = list(range(t0 // 128, (t0 + wg) // 128))
                    else:
                        tts = list(range(ntt))
                    for i in range(0, len(tts), 2):
                        pair = tts[i : i + 2]
                        pss = base_pair(pair, oc)
                        if pending is not None:
                            lora_out_pair(*pending, oc)
                        pending = (pair, pss)
                if pending is not None:
                    lora_out_pair(*pending, oc)

            # Sweep 1: oc0 half of every tile (needs only W-oc0 early);
            # Sweep 2: oc1 halves — W-oc1 had all of sweep 1 to land.
            sweep(0)
            sweep(1)

    return nc


def _get_compiled(cap: int):
    if cap not in _compiled_cache:
        _compiled_cache[cap] = _build(cap)
    return _compiled_cache[cap]


# ---------------------------------------------------------------------------
# Host-side routing + execution
# ---------------------------------------------------------------------------


def _reference_numpy(x, group_sizes, adapter_indices_sorted, weight, lora_A, lora_B, lora_scaling):
    """Fallback replicating the jax reference exactly (only used for
    degenerate group_sizes that do not sum to T)."""
    x = np.asarray(x, np.float32)
    gs = np.asarray(group_sizes, np.int64)
    adapter = np.asarray(adapter_indices_sorted, np.int64)
    out = np.zeros((x.shape[0], weight.shape[2]), np.float32)
    # base: ragged_dot semantics (groups from cumsum, tail rows -> 0)
    offs = np.minimum(np.concatenate([[0], np.cumsum(gs)]), x.shape[0])
    for e in range(E):
        s, t = offs[e], offs[e + 1]
        if t > s:
            out[s:t] = x[s:t] @ weight[e]
    # lora: expert ids via repeat padded with the final value
    rep = np.repeat(np.arange(E), np.maximum(gs, 0))[: x.shape[0]]
    if rep.size == 0:
        rep = np.zeros(x.shape[0], np.int64)
    elif rep.size < x.shape[0]:
        rep = np.concatenate(
            [rep, np.full(x.shape[0] - rep.size, rep[-1], np.int64)]
        )
    for t in range(x.shape[0]):
        e, a = rep[t], adapter[t]
        inter = x[t] @ lora_A[a, e]
        out[t] += lora_scaling[a] * (inter @ lora_B[a, e])
    return out


def kernel(x, group_sizes, adapter_indices_sorted, weight, lora_A, lora_B, lora_scaling):
    import ml_dtypes

    x = np.ascontiguousarray(np.asarray(x, np.float32))
    weight = np.asarray(weight, np.float32)
    lora_A = np.asarray(lora_A, np.float32)
    lora_B = np.asarray(lora_B, np.float32)
    scaling = np.asarray(lora_scaling, np.float32)
    gs = np.asarray(group_sizes).astype(np.int64)
    adapter = np.asarray(adapter_indices_sorted).astype(np.int64)

    if gs.sum() != T or (gs < 0).any():
        return _reference_numpy(
            x, gs, adapter, weight, lora_A, lora_B, scaling
        )

    from concourse.bass_utils import run_bass_kernel_spmd

    bf = ml_dtypes.bfloat16
    f8 = ml_dtypes.float8_e4m3
    cap = int(max(128, -(-int(gs.max()) // 128) * 128))
    nc = _get_compiled(cap)
    KC2 = KC // 2

    offs = np.concatenate([[0], np.cumsum(gs)])
    in_maps = []
    for e in range(NCORES):
        n = int(gs[e])
        s = int(offs[e])
        ngr = (cap + 511) // 512
        xe = np.zeros((ngr * 512, IN), np.float32)
        xe[:n] = x[s : s + n]
        # XT[g, p, tb*KC*128 + k*128 + c] = x_e[512g+128tb+c, 128k+p]
        xt = np.ascontiguousarray(
            xe.reshape(ngr, 4, 128, KC, 128)
            .transpose(0, 4, 1, 3, 2)
            .reshape(ngr, 128, 4 * KC * 128)
            .astype(bf)
        )
        # X8[g, p, (kc*2+i)*512+c] = fp8(x_e[512g+c, 256kc+128i+p])
        x8 = np.ascontiguousarray(
            xe.reshape(ngr, 512, KC2, 2, 128)
            .transpose(0, 4, 2, 3, 1)
            .reshape(ngr, 128, KC2 * 2 * 512)
            .astype(f8)
        )
        # W2[p, oc, k, c] = weight[e][128k+p, 512oc+c]
        w = np.ascontiguousarray(
            weight[e].reshape(KC, 128, OC, 512).transpose(1, 2, 0, 3).astype(bf)
        )
        # A_cat[:, a*R+r] = lora_A[a, e, :, r]; A8[p, kc, i, j] =
        # fp8(A_cat[256kc+128i+p, j])
        acat_full = lora_A[:, e].transpose(1, 0, 2).reshape(IN, AR)
        a8 = np.ascontiguousarray(
            acat_full.reshape(KC2, 2, 128, AR).transpose(2, 0, 1, 3).astype(f8)
        )
        bcat = np.ascontiguousarray(lora_B[:, e].reshape(AR, OUT).astype(bf))
        ae = adapter[s : s + n]
        m = np.zeros((A, cap), np.float32)
        m[ae, np.arange(n)] = scaling[ae]
        maskt = np.ascontiguousarray(np.repeat(m, R, axis=0).astype(bf))
        in_maps.append(
            {"xt": xt, "x8": x8, "w": w, "a8": a8, "bcat": bcat, "maskt": maskt}
        )

    res = run_bass_kernel_spmd(nc, in_maps, list(range(NCORES)))

    out = np.empty((T, OUT), np.float32)
    for e in range(NCORES):
        n = int(gs[e])
        if n:
            out[int(offs[e]) : int(offs[e]) + n] = (
                res.results[e]["out"][:n].astype(np.float32)
            )
    return out


# revision 26
# speedup vs baseline: 1.1485x; 1.0019x over previous
"""Trainium2 Bass kernel for nn_LoRAExpert (moe_routing).

Per token t (expert e_t from contiguous group_sizes, adapter a_t):

    out[t] = x[t] @ W[e_t] + s_{a_t} * (x[t] @ A[a_t, e_t]) @ B[a_t, e_t]

Strategy (expert-parallel over 8 NeuronCores):
  - Host routes tokens: x is already expert-sorted, so core e gets the
    contiguous slice x[off_e : off_e + gs_e], padded to a common `cap`.
  - LoRA routing trick: with A=8 adapters and rank R=16, the per-expert
    concatenation A_cat = [A[0,e] .. A[7,e]] is [1024, 128]. Compute
    inter_all = x @ A_cat densely for ALL adapters, then multiply by a
    per-token mask M[j, t] = s_{a_t} * (j in adapter-a_t block) and feed
    the masked inter into B_cat = [B[0,e]; ..; B[7,e]] ([128, 1024]).
    This turns the ragged adapter grouping into two dense matmuls and
    one elementwise mask — no on-device sorting or control flow.
  - The B-side matmul accumulates into the same PSUM tile as the base
    matmul, so base + lora is free.
  - All matmul operands are cast to bf16 on the host (fp32 PSUM
    accumulation on the PE); output is written bf16 and widened to fp32
    on the host (absmax rel-err budget is 2e-2; this costs ~3e-3).
  - DMA plan: all inputs go on the Sync HWDGE queue in priority order;
    the output tiles go on the Scalar HWDGE queue so they never block
    late x groups. PSUM->SBUF casts split across Vector (oc0) and
    Scalar (oc1).
  - The base+lora matmuls run as two output-column sweeps: sweep 1
    computes the first 512 output columns of every token tile, sweep 2
    the second 512. Sweep 1 only needs the oc0 half of W (1 MB) at the
    head, at a demand rate (~70 B/ns) far below queue bandwidth, so the
    PE never outruns the input stream; W-oc1 has ~35us to arrive.
  - Phase 1 (x8 @ A8) runs in fp8 DoubleRow (256-row contraction), so
    it depends only on the small fp8 x stream, not the bulk bf16 x.

The kernel is compiled for cap = max(group_sizes) rounded up to 128 and
cached per cap. All 8 cores run one SPMD program; per-core data differs
only through the input maps.
"""

import numpy as np

T, E, IN, OUT, A, R = 16384, 8, 1024, 1024, 8, 16
NCORES = 8
AR = A * R  # 128
KC = IN // 128  # 8 contraction chunks
OC = OUT // 512  # 2 output column chunks

_compiled_cache: dict[int, object] = {}


# ---------------------------------------------------------------------------
# walrus in this container accepts at most 1 sync-wait command per
# instruction; Tile attaches more. Split excess waits onto no-ops.
# ---------------------------------------------------------------------------


def _apply_tile_wait_patch():
    import bass_rust
    import concourse.tile as tile
    from concourse import mybir
    from concourse.vector_clock import ScopedClock

    if getattr(tile.TileContext, "_wait_split_patched", False):
        return

    MAX_WAITS = 1

    def _split_excess_waits(nc):
        for fn in nc.m.functions:
            for blk in fn.blocks:
                insts = blk.instructions  # live list
                i = 0
                while i < len(insts):
                    inst = insts[i]
                    si = inst.sync_info
                    if si is not None and len(si.on_wait) > MAX_WAITS:
                        waits = list(si.on_wait)
                        keep = waits[-MAX_WAITS:]
                        excess = waits[:-MAX_WAITS]
                        inst.sync_info = bass_rust.SyncInfo(
                            on_wait=keep, on_update=list(si.on_update)
                        )
                        pos = i
                        for k in range(0, len(excess), MAX_WAITS):
                            nop = mybir.InstNoOp(
                                name=f"{inst.name}-hoistw{k}",
                                engine=inst.engine,
                                bass_nofuse=True,
                                sync_info=mybir.SyncInfo(
                                    on_wait=excess[k : k + MAX_WAITS], on_update=[]
                                ),
                            )
                            insts.insert(pos, nop)
                            pos += 1
                            i += 1
                    i += 1

    def _split_drain_and_barrier(self, tick_clock, wait_clock):
        nc = self.nc
        drain_inst = nc.sync.drain()
        wait_clock.add_sem_waits(
            drain_inst.ins, ScopedClock({None: tick_clock.global_clock})
        )
        si = drain_inst.ins.sync_info
        if si is not None and len(si.on_wait) > MAX_WAITS:
            waits = list(si.on_wait)
            drain_inst.ins.sync_info = bass_rust.SyncInfo(
                on_wait=waits[:MAX_WAITS], on_update=list(si.on_update)
            )
            for k in range(MAX_WAITS, len(waits), MAX_WAITS):
                extra = nc.sync.drain()
                extra.ins.sync_info = bass_rust.SyncInfo(
                    on_wait=waits[k : k + MAX_WAITS], on_update=[]
                )

        import os as _os

        nc.all_engine_barrier()
        assert self.sems is not None
        popped = nc._tile_sem_poison_stack.pop()
        assert popped is self._sem_poison
        nc.clear_and_free_semaphores(list(self.sems.allocated().values()))
        if _os.environ.get("LORA_LEAN_TAIL", "1") != "1":
            # Second barrier only matters for kernels that continue past
            # the TileContext; ours ends here (sem clears trail on gpsimd).
            nc.all_engine_barrier()

        _split_excess_waits(nc)

    tile.TileContext._drain_and_barrier = _split_drain_and_barrier
    tile.TileContext._wait_split_patched = True


# ---------------------------------------------------------------------------
# Bass program (one SPMD NeuronCore program, parameterized by cap)
# ---------------------------------------------------------------------------


def _build(cap: int):
    import concourse.bass as bass
    import concourse.tile as tile
    from concourse import mybir

    _apply_tile_wait_patch()

    ntt = cap // 128  # token tiles
    ngr = (cap + 511) // 512  # inter groups of up to 512 tokens

    bf16 = mybir.dt.bfloat16
    f32 = mybir.dt.float32

    f8 = mybir.dt.float8e4
    KC2 = KC // 2  # fp8 DoubleRow contracts 256 rows per instruction

    NTB = 512 // 128  # token blocks per group
    TBW = KC * 128  # columns per token block in XT

    nc = bass.Bass()
    # XT[g, p, tb*KC*128 + k*128 + c] = x_e[512g + 128tb + c, 128k + p]
    # (token-block-major so a single token tile's x is one contiguous run)
    XT = nc.dram_tensor("xt", [ngr, 128, NTB * TBW], bf16, kind="ExternalInput")
    # X8[g, p, ((kc*2+i)*512)+c] = fp8(x_e[512g + c, 256kc + 128i + p])
    X8 = nc.dram_tensor("x8", [ngr, 128, KC2 * 2 * 512], f8, kind="ExternalInput")
    # W2[p, oc, k, c] = weight[128k + p, 512oc + c] (oc-major halves)
    W = nc.dram_tensor("w", [128, OC, KC, 512], bf16, kind="ExternalInput")
    A8T = nc.dram_tensor("a8", [128, KC2, 2, AR], f8, kind="ExternalInput")
    BCAT = nc.dram_tensor("bcat", [AR, OUT], bf16, kind="ExternalInput")
    MASKT = nc.dram_tensor("maskt", [AR, cap], bf16, kind="ExternalInput")
    OUTD = nc.dram_tensor("out", [cap, OUT], bf16, kind="ExternalOutput")

    def gslice(g):
        t0 = g * 512
        return t0, min(512, cap - t0)

    with tile.TileContext(nc) as tc:
        with (
            tc.tile_pool(name="big", bufs=1) as big,
            tc.tile_pool(name="outp", bufs=4) as outp,
            tc.tile_pool(name="psi", bufs=2, space="PSUM") as psi,
            tc.tile_pool(name="pso", bufs=4, space="PSUM") as pso,
        ):
            # Warm the PE's HAM clock gate during the input-DMA lead-in:
            # ~4us of matmuls on a zeroed tile so real matmuls start at
            # 2.4 GHz instead of 1.2 GHz. Targets a psi bank (all 8 PSUM
            # banks are accounted: 6 pso + 2 psi).
            warm_sb = big.tile([128, 512], bf16)
            nc.vector.memset(warm_sb[:], 0.0)
            wps = psi.tile([128, 512], f32, name="warm", tag="psi")
            NWARM = 20  # sized to span the head input-DMA window so the
            for i in range(NWARM):  # clock is fully ramped at first real work
                nc.tensor.matmul(
                    wps[:], warm_sb[:, 0:128], warm_sb[:],
                    start=(i == 0), stop=(i == NWARM - 1),
                )
            # All inputs on the Sync HWDGE queue, ordered so each item
            # lands just before the PE needs it in the oc0 sweep; the
            # scalar queue only carries output tiles (emitted later).
            a8_sb = big.tile([128, KC2, 2, AR], f8)
            x8_sb = big.tile([128, ngr, KC2, 2, 512], f8)
            xt_sb = big.tile([128, ngr, NTB, KC, 128], bf16)
            w_sb = big.tile([128, OC, KC, 512], bf16)
            maskt_sb = big.tile([AR, cap], bf16)
            b_sb = big.tile([AR, OUT], bf16)
            nc.sync.dma_start(a8_sb[:], A8T[:])
            nc.sync.dma_start(x8_sb[:, 0, 0:2], X8[0, :, 0 : 2 * 1024])
            nc.sync.dma_start(x8_sb[:, 0, 2:4], X8[0, :, 2 * 1024 : 4 * 1024])
            nc.sync.dma_start(w_sb[:, 0, 0:2, :], W[:, 0, 0:2, :])
            nc.sync.dma_start(xt_sb[:, 0, 0], XT[0, :, 0:TBW])
            nc.sync.dma_start(xt_sb[:, 0, 1], XT[0, :, TBW : 2 * TBW])
            nc.sync.dma_start(w_sb[:, 0, 2:4, :], W[:, 0, 2:4, :])
            nc.sync.dma_start(w_sb[:, 0, 4:6, :], W[:, 0, 4:6, :])
            nc.sync.dma_start(w_sb[:, 0, 6:8, :], W[:, 0, 6:8, :])
            nc.sync.dma_start(maskt_sb[:], MASKT[:])
            nc.sync.dma_start(b_sb[:], BCAT[:])
            nc.sync.dma_start(xt_sb[:, 0, 2], XT[0, :, 2 * TBW : 3 * TBW])
            if ngr > 1:
                nc.sync.dma_start(x8_sb[:, 1], X8[1])
            nc.sync.dma_start(xt_sb[:, 0, 3], XT[0, :, 3 * TBW : 4 * TBW])
            if ngr > 1:
                nc.sync.dma_start(xt_sb[:, 1, 0:2], XT[1, :, 0 : 2 * TBW])
                nc.sync.dma_start(xt_sb[:, 1, 2:4], XT[1, :, 2 * TBW : 4 * TBW])
            if ngr > 2:
                nc.sync.dma_start(x8_sb[:, 2], X8[2])
                nc.sync.dma_start(xt_sb[:, 2], XT[2])
            if ngr > 3:
                nc.sync.dma_start(x8_sb[:, 3], X8[3])
                nc.sync.dma_start(xt_sb[:, 3], XT[3])
            # W-oc1 pieces next: needed from sweep 2 (~2/3 into the run)
            # but must not sit behind the final x groups.
            nc.sync.dma_start(w_sb[:, 1, 0:2, :], W[:, 1, 0:2, :])
            nc.sync.dma_start(w_sb[:, 1, 2:4, :], W[:, 1, 2:4, :])
            for g in range(4, ngr):
                nc.sync.dma_start(x8_sb[:, g], X8[g])
                nc.sync.dma_start(xt_sb[:, g], XT[g])
            nc.sync.dma_start(w_sb[:, 1, 4:6, :], W[:, 1, 4:6, :])
            nc.sync.dma_start(w_sb[:, 1, 6:8, :], W[:, 1, 6:8, :])

            interm_sb = big.tile([AR, cap], bf16)

            def phase1(g):
                # inter_all = (x8 @ A8_cat)^T for group g (fp8 DoubleRow,
                # 256-row contraction per matmul), masked -> interm_sb
                t0, wg = gslice(g)
                ps = psi.tile([128, 512], f32, name=f"psi{g}", tag="psi")
                for kc in range(KC2):
                    nc.tensor.matmul(
                        ps[:, :wg],
                        a8_sb[:, kc],
                        x8_sb[:, g, kc, :, 0:wg],
                        start=(kc == 0),
                        stop=(kc == KC2 - 1),
                        perf_mode=mybir.MatmulPerfMode.DoubleRow,
                    )
                nc.vector.scalar_tensor_tensor(
                    interm_sb[:, t0 : t0 + wg],
                    ps[:, :wg],
                    1.0,
                    maskt_sb[:, t0 : t0 + wg],
                    mybir.AluOpType.mult,
                    mybir.AluOpType.mult,
                )

            def base_pair(tts, oc):
                # base k-loops for up to two token tiles, interleaved so
                # consecutive matmuls hit different PSUM banks (same-bank
                # back-to-back accumulation costs ~24ns/matmul).
                pss = [
                    pso.tile([128, 512], f32, name=f"ps{tt}_{oc}", tag="pso")
                    for tt in tts
                ]
                for k in range(KC):
                    for ps, tt in zip(pss, tts):
                        ts0 = tt * 128
                        g, tb = ts0 // 512, (ts0 % 512) // 128
                        nc.tensor.matmul(
                            ps[:],
                            xt_sb[:, g, tb, k, :],
                            w_sb[:, oc, k, :],
                            start=(k == 0),
                            stop=False,
                        )
                return pss

            def lora_out_pair(tts, pss, oc):
                # lora matmuls for the pair, then cast + per-half DMA out.
                for ps, tt in zip(pss, tts):
                    ts0 = tt * 128
                    nc.tensor.matmul(
                        ps[:],
                        interm_sb[:, ts0 : ts0 + 128],
                        b_sb[:, oc * 512 : oc * 512 + 512],
                        start=False,
                        stop=True,
                    )
                for ps, tt in zip(pss, tts):
                    ts0 = tt * 128
                    o_sb = outp.tile([128, 512], bf16, name=f"o{tt}_{oc}", tag="outp")
                    # alternate cast engines by tile parity so a pair's
                    # two casts run concurrently (vector + scalar)
                    if tt % 2 == 0:
                        nc.vector.tensor_copy(o_sb[:], ps[:])
                    else:
                        nc.scalar.copy(o_sb[:], ps[:])
                    nc.scalar.dma_start(
                        OUTD[ts0 : ts0 + 128, oc * 512 : oc * 512 + 512], o_sb[:]
                    )

            def sweep(oc):
                # software-pipelined: pair i's lora/copy runs after pair
                # i+1's base k-loop, giving mask/B/STT extra slack.
                pending = None
                for g in range(ngr) if oc == 0 else [None]:
                    if oc == 0:
                        phase1(g)
                        t0, wg = gslice(g)
                        tts = list(range(t0 // 128, (t0 + wg) // 128))
                    else:
                        tts = list(range(ntt))
                    for i in range(0, len(tts), 2):
                        pair = tts[i : i + 2]
                        pss = base_pair(pair, oc)
                        if pending is not None:
                            lora_out_pair(*pending, oc)
                        pending = (pair, pss)
                if pending is not None:
                    lora_out_pair(*pending, oc)

            # Sweep 1: oc0 half of every tile (needs only W-oc0 early);
            # Sweep 2: oc1 halves — W-oc1 had all of sweep 1 to land.
            sweep(0)
            sweep(1)

    return nc


def _get_compiled(cap: int):
    if cap not in _compiled_cache:
        _compiled_cache[cap] = _build(cap)
    return _compiled_cache[cap]


# ---------------------------------------------------------------------------
# Host-side routing + execution
# ---------------------------------------------------------------------------


def _reference_numpy(x, group_sizes, adapter_indices_sorted, weight, lora_A, lora_B, lora_scaling):
    """Fallback replicating the jax reference exactly (only used for
    degenerate group_sizes that do not sum to T)."""
    x = np.asarray(x, np.float32)
    gs = np.asarray(group_sizes, np.int64)
    adapter = np.asarray(adapter_indices_sorted, np.int64)
    out = np.zeros((x.shape[0], weight.shape[2]), np.float32)
    # base: ragged_dot semantics (groups from cumsum, tail rows -> 0)
    offs = np.minimum(np.concatenate([[0], np.cumsum(gs)]), x.shape[0])
    for e in range(E):
        s, t = offs[e], offs[e + 1]
        if t > s:
            out[s:t] = x[s:t] @ weight[e]
    # lora: expert ids via repeat padded with the final value
    rep = np.repeat(np.arange(E), np.maximum(gs, 0))[: x.shape[0]]
    if rep.size == 0:
        rep = np.zeros(x.shape[0], np.int64)
    elif rep.size < x.shape[0]:
        rep = np.concatenate(
            [rep, np.full(x.shape[0] - rep.size, rep[-1], np.int64)]
        )
    for t in range(x.shape[0]):
        e, a = rep[t], adapter[t]
        inter = x[t] @ lora_A[a, e]
        out[t] += lora_scaling[a] * (inter @ lora_B[a, e])
    return out


def kernel(x, group_sizes, adapter_indices_sorted, weight, lora_A, lora_B, lora_scaling):
    import ml_dtypes

    x = np.ascontiguousarray(np.asarray(x, np.float32))
    weight = np.asarray(weight, np.float32)
    lora_A = np.asarray(lora_A, np.float32)
    lora_B = np.asarray(lora_B, np.float32)
    scaling = np.asarray(lora_scaling, np.float32)
    gs = np.asarray(group_sizes).astype(np.int64)
    adapter = np.asarray(adapter_indices_sorted).astype(np.int64)

    if gs.sum() != T or (gs < 0).any():
        return _reference_numpy(
            x, gs, adapter, weight, lora_A, lora_B, scaling
        )

    from concourse.bass_utils import run_bass_kernel_spmd

    bf = ml_dtypes.bfloat16
    f8 = ml_dtypes.float8_e4m3
    cap = int(max(128, -(-int(gs.max()) // 128) * 128))
    nc = _get_compiled(cap)
    KC2 = KC // 2

    offs = np.concatenate([[0], np.cumsum(gs)])
    in_maps = []
    for e in range(NCORES):
        n = int(gs[e])
        s = int(offs[e])
        ngr = (cap + 511) // 512
        xe = np.zeros((ngr * 512, IN), np.float32)
        xe[:n] = x[s : s + n]
        # XT[g, p, tb*KC*128 + k*128 + c] = x_e[512g+128tb+c, 128k+p]
        xt = np.ascontiguousarray(
            xe.reshape(ngr, 4, 128, KC, 128)
            .transpose(0, 4, 1, 3, 2)
            .reshape(ngr, 128, 4 * KC * 128)
            .astype(bf)
        )
        # X8[g, p, (kc*2+i)*512+c] = fp8(x_e[512g+c, 256kc+128i+p])
        x8 = np.ascontiguousarray(
            xe.reshape(ngr, 512, KC2, 2, 128)
            .transpose(0, 4, 2, 3, 1)
            .reshape(ngr, 128, KC2 * 2 * 512)
            .astype(f8)
        )
        # W2[p, oc, k, c] = weight[e][128k+p, 512oc+c]
        w = np.ascontiguousarray(
            weight[e].reshape(KC, 128, OC, 512).transpose(1, 2, 0, 3).astype(bf)
        )
        # A_cat[:, a*R+r] = lora_A[a, e, :, r]; A8[p, kc, i, j] =
        # fp8(A_cat[256kc+128i+p, j])
        acat_full = lora_A[:, e].transpose(1, 0, 2).reshape(IN, AR)
        a8 = np.ascontiguousarray(
            acat_full.reshape(KC2, 2, 128, AR).transpose(2, 0, 1, 3).astype(f8)
        )
        bcat = np.ascontiguousarray(lora_B[:, e].reshape(AR, OUT).astype(bf))
        ae = adapter[s : s + n]
        m = np.zeros((A, cap), np.float32)
        m[ae, np.arange(n)] = scaling[ae]
        maskt = np.ascontiguousarray(np.repeat(m, R, axis=0).astype(bf))
        in_maps.append(
            {"xt": xt, "x8": x8, "w": w, "a8": a8, "bcat": bcat, "maskt": maskt}
        )

    res = run_bass_kernel_spmd(nc, in_maps, list(range(NCORES)))

    out = np.empty((T, OUT), np.float32)
    for e in range(NCORES):
        n = int(gs[e])
        if n:
            out[int(offs[e]) : int(offs[e]) + n] = (
                res.results[e]["out"][:n].astype(np.float32)
            )
    return out
